# revision 20
# baseline (speedup 1.0000x reference)
"""Distillation-trainer loss kernel for Trainium2 (8 NeuronCores).

Computes  loss = mean((attn(q,k,v) - attn(q,ck,cv))**2)  for
q:[1,8,1024,128], k/v:[1,8,8192,128], ck/cv:[1,8,1024,128] fp32.

Sharding: one kv-head per core (h axis, 8 heads / 8 cores). Each core
returns its head's unnormalized attention outputs + softmax sums; the
host normalizes and reduces the scalar loss (the "all-reduce").

Per-core algorithm (head h):
  - K/CK/Q PE-transposed (bf16) to [d, n] / [d, q] layouts.
  - scoresT[n-tile, 0:1024] : stationary = kT tile, moving = full qT
    (2 matmuls of N=512). One LDWEIGHTS per n-tile amortized over 1024
    moving columns (v1 was weight-port-bound with 944 LDWEIGHTS).
  - exp on ACT -> fp8e4 probsT ring [n, q], one 1024-elem call per
    tile. exp bias -3.7 cancels in the softmax but keeps exp() < 224:
    the HW fp8e4 converter overflows to inf above ~240 (not 448 like
    ml_dtypes in CoreSim) and the max q.k/sqrt(d) score in this data
    is ~8.6 (dot-product tails are heavier than Gaussian).
  - PV in fp8 DoubleRow (2 n-tiles per matmul, contraction 256):
    stationary = V' [n, 2, 80] where cols 0:64 = V d-half, col 64 =
    ones (softmax denominator accumulates in PSUM row 64), 65:80 zero
    pad (DoubleRow k-tile byte step must be %16). probsT is the
    moving operand -- weight port stays far under the moving port.
  - unnormalized z' and S are DMA'd out; the host does z'/S and the
    MSE (on-device normalize cost ~26us of serial tail).
  - scheduling: PV lags one pair behind exp so the PE always refills
    the freed scores slot first (the exp chain never starves); K
    transposes are spread through the teacher loop at t%8==1.
"""

import numpy as np

import concourse.bass as bass
import concourse.mybir as mybir
import concourse.tile as tile
from concourse import bacc
from concourse.masks import make_identity
from concourse.bass_utils import run_bass_kernel_spmd

F32 = mybir.dt.float32
BF16 = mybir.dt.bfloat16
FP8 = mybir.dt.float8e4     # e4m3: PV operands (exp probs, values)
AF = mybir.ActivationFunctionType
ALU = mybir.AluOpType
DR = mybir.MatmulPerfMode.DoubleRow

B, H, Q, N, NC, D = 1, 8, 1024, 8192, 1024, 128
N_CORES = 8
SCALE = 1.0 / float(np.sqrt(D))
EXP_BIAS = -3.7

NT = N // 128               # 64 teacher n-tiles
NCT = NC // 128             # 8 compressed n-tiles
VW = 80                     # DoubleRow stationary width: 64 V + ones + pad
QC = 512                    # q chunk (PSUM bank = 512 fp32)
RING = 8                    # probsT ring depth (tiles)


def _emit(nc: bass.Bass, tc: tile.TileContext, qh, kh, vh, ckh, cvh,
          za_out, zb_out):
    ctxs = []

    def pool(**kw):
        p = tc.tile_pool(**kw)
        ctxs.append(p)
        return p.__enter__()

    pconst = pool(name="pconst", bufs=1)
    pstage = pool(name="pstage", bufs=2)
    psmall = pool(name="psmall", bufs=4)
    psc = pool(name="psc", bufs=2, space="PSUM")   # scores / transpose scratch
    ppv = pool(name="ppv", bufs=1, space="PSUM")   # PV accumulators

    # ---- persistent SBUF tensors ----
    ident = pconst.tile([128, 128], BF16, tag="ident")
    make_identity(nc, ident[:])

    qT = pconst.tile([128, Q], BF16, tag="qT")             # [d, q]
    kT = pconst.tile([128, NT, 128], BF16, tag="kT")       # [d, t, n]
    ckT = pconst.tile([128, NCT, 128], BF16, tag="ckT")
    va = pconst.tile([128, NT // 2, 2, VW], FP8, tag="va")   # V[:, :, 0:64]|1|0
    vb = pconst.tile([128, NT // 2, 2, VW], FP8, tag="vb")   # V[:, :, 64:128]|0
    cva = pconst.tile([128, NCT // 2, 2, VW], FP8, tag="cva")
    cvb = pconst.tile([128, NCT // 2, 2, VW], FP8, tag="cvb")
    ring = pconst.tile([128, RING, Q], FP8, tag="ring")    # probsT ring [n, q]

    for t_ in (va, cva):
        nc.gpsimd.memset(t_[:, :, :, 64:65], 1.0)
        nc.gpsimd.memset(t_[:, :, :, 65:VW], 0.0)
    for t_ in (vb, cvb):
        nc.gpsimd.memset(t_[:, :, :, 64:VW], 0.0)

    ebias = pconst.tile([128, 1], F32, tag="ebias")
    nc.gpsimd.memset(ebias[:], EXP_BIAS)

    # Warm the ACT exp table while prep DMAs run (~2.7us ACT_TABLE_LOAD).
    warm = psmall.tile([128, 1], F32, tag="warm")
    nc.gpsimd.memset(warm[:], 0.0)
    warm2 = psmall.tile([128, 1], F32, tag="warm2")
    nc.scalar.activation(warm2[:], warm[:], AF.Exp)

    # ---- loaders ----
    def load_kT_chunk(src, dst, g, tag):
        # 1024 rows -> cast bf16 -> 8 PE transposes -> dst[:, 8g:8g+8, :]
        stg = pstage.tile([128, 8, 128], F32, tag=tag)
        ap = src[g * 1024:(g + 1) * 1024, :].rearrange("(i p) d -> p i d", p=128)
        nc.sync.dma_start(out=stg[:], in_=ap)
        kb = pstage.tile([128, 8, 128], BF16, tag=tag + "b")
        nc.vector.tensor_copy(kb[:], stg[:])
        tp = psc.tile([128, 8, 128], BF16, tag="sc")
        for j in range(8):
            nc.tensor.transpose(tp[:, j, :], kb[:, j, :], ident[:])
        nc.vector.tensor_copy(dst[:, 8 * g:8 * g + 8, :], tp[:])

    def load_v_chunk(src, dsta, dstb, g, tag):
        # 1024 rows of V -> pairs 4g..4g+3, split d halves, cast to fp8.
        stg = pstage.tile([128, 8, 128], F32, tag=tag)
        ap = src[g * 1024:(g + 1) * 1024, :].rearrange("(i p) d -> p i d", p=128)
        nc.sync.dma_start(out=stg[:], in_=ap)
        sv = stg[:].rearrange("p (a b) d -> p a b d", b=2)  # [128, 4, 2, 128]
        nc.vector.tensor_copy(dsta[:, 4 * g:4 * g + 4, :, 0:64], sv[:, :, :, 0:64])
        nc.vector.tensor_copy(dstb[:, 4 * g:4 * g + 4, :, 0:64], sv[:, :, :, 64:128])

    load_kT_chunk(qh, qT[:].rearrange("p (i n) -> p i n", i=8), 0, "stq")
    load_kT_chunk(ckh, ckT, 0, "stck")
    load_v_chunk(cvh, cva, cvb, 0, "stcv")

    # K chunk DMAs into persistent fp32 staging; transposes happen later,
    # spread through the teacher loop. K/V interleaved so both arrive in
    # tile order on the serial DMA queue.
    kstg = pconst.tile([128, NT, 128], F32, tag="kstg")
    kb16 = pconst.tile([128, NT, 128], BF16, tag="kb16")

    def dma_k_chunk(g):
        kap = kh[g * 1024:(g + 1) * 1024, :].rearrange("(i p) d -> p i d", p=128)
        nc.sync.dma_start(out=kstg[:, 8 * g:8 * g + 8, :], in_=kap)
        nc.vector.tensor_copy(kb16[:, 8 * g:8 * g + 8, :], kstg[:, 8 * g:8 * g + 8, :])

    def transpose_k_chunk(g):
        tp = psc.tile([128, 8, 128], BF16, tag="sc")
        for j in range(8):
            nc.tensor.transpose(tp[:, j, :], kb16[:, 8 * g + j, :], ident[:])
        nc.vector.tensor_copy(kT[:, 8 * g:8 * g + 8, :], tp[:])

    for g in range(NT // 8):
        dma_k_chunk(g)
        load_v_chunk(vh, va, vb, g, "stv")

    # ---- PV accumulators (persist across one phase) ----
    za = [ppv.tile([128, QC], F32, tag=f"za{i}", name=f"za{i}") for i in range(2)]
    zb = [ppv.tile([128, QC], F32, tag=f"zb{i}", name=f"zb{i}") for i in range(2)]

    # Schraudolph fast-exp constants for the DVE offload path:
    # exp(s*SCALE + EXP_BIAS) ~ bitcast_f32(int32(A*s + B)). Max rel err
    # ~3%, comparable to the fp8e4 quantization the probs get anyway.
    LOG2E = 1.4426950408889634
    SCHRA_A = float(2.0 ** 23 * LOG2E * SCALE)
    SCHRA_B = float(2.0 ** 23 * (127 - 0.0436) + 2.0 ** 23 * LOG2E * EXP_BIAS)
    I32 = mybir.dt.int32

    def attend_tile(keysT, t, slot, dve_exp=False):
        sc = psc.tile([128, 2, QC], F32, tag="sc")
        nc.tensor.matmul(sc[:, 0, :], keysT[:, t, :], qT[:, 0:QC],
                         start=True, stop=True)
        nc.tensor.matmul(sc[:, 1, :], keysT[:, t, :], qT[:, QC:Q],
                         start=True, stop=True)
        if dve_exp:
            # exp on the (teacher-phase-idle) DVE so the ACT chain skips
            # this tile; only pass 1 holds the PSUM slot.
            ti = psmall.tile([128, Q], I32, tag="ti")
            nc.vector.tensor_scalar(ti[:], sc[:].rearrange("p a b -> p (a b)"),
                                    SCHRA_A, SCHRA_B, ALU.mult, ALU.add)
            nc.vector.tensor_copy(ring[:, slot, :], ti[:].bitcast(F32))
        else:
            ex = ring[:, slot, :].rearrange("p (a b) -> p a b", a=2)
            nc.scalar.activation(ex, sc[:], AF.Exp, scale=SCALE, bias=ebias[:])

    def pv_pair(vsa, vsb, p, n_pairs, slot0):
        st = dict(start=(p == 0), stop=(p == n_pairs - 1))
        rr = ring[:, slot0:slot0 + 2, :]   # [128, 2, 1024]
        for qc in range(2):
            mv = rr[:, :, qc * QC:(qc + 1) * QC]
            nc.tensor.matmul(za[qc][0:VW, :], vsa[:, p, :, :], mv,
                             perf_mode=DR, **st)
            nc.tensor.matmul(zb[qc][0:VW, :], vsb[:, p, :, :], mv,
                             perf_mode=DR, **st)

    def dump_phase(phase):
        # za rows 0:64 = z' d 0:64, row 64 = S; zb rows 0:64 = z' d 64:128
        for qc in range(2):
            da = psmall.tile([65, QC], F32, tag="da")
            nc.vector.tensor_copy(da[:], za[qc][0:65, :])
            nc.sync.dma_start(out=za_out[:, phase, qc, :], in_=da[:])
            db = psmall.tile([64, QC], F32, tag="db")
            nc.vector.tensor_copy(db[:], zb[qc][0:64, :])
            nc.sync.dma_start(out=zb_out[:, phase, qc, :], in_=db[:])

    # ---- Phase 1: compressed attention ----
    for t in range(NCT):
        attend_tile(ckT, t, t)
        if t % 2 == 1 and t >= 3:
            pv_pair(cva, cvb, (t - 2) // 2, NCT // 2, t - 3)
    pv_pair(cva, cvb, NCT // 2 - 1, NCT // 2, NCT - 2)
    transpose_k_chunk(0)
    dump_phase(0)

    # ---- Phase 2: teacher attention (PV lags one pair so the PE can
    # always refill the freed scores slot before stalling on exp) ----
    for t in range(NT):
        attend_tile(kT, t, t % RING, dve_exp=(t % 4 == 2))
        if t % 2 == 1 and t >= 3:
            pv_pair(va, vb, (t - 2) // 2, NT // 2, (t - 3) % RING)
        if t % 8 == 1 and t // 8 < 7:
            transpose_k_chunk(t // 8 + 1)
    pv_pair(va, vb, NT // 2 - 1, NT // 2, (NT - 2) % RING)
    dump_phase(1)

    for p in reversed(ctxs):
        p.__exit__(None, None, None)


_NC_CACHE = None


def build_nc():
    global _NC_CACHE
    if _NC_CACHE is not None:
        return _NC_CACHE
    nc = bacc.Bacc()
    qh = nc.declare_dram_parameter("queries", [Q, D], F32, isOutput=False)
    kh = nc.declare_dram_parameter("keys", [N, D], F32, isOutput=False)
    vh = nc.declare_dram_parameter("values", [N, D], F32, isOutput=False)
    ckh = nc.declare_dram_parameter("c_keys", [NC, D], F32, isOutput=False)
    cvh = nc.declare_dram_parameter("c_values", [NC, D], F32, isOutput=False)
    za_out = nc.declare_dram_parameter("za_out", [65, 2, 2, QC], F32, isOutput=True)
    zb_out = nc.declare_dram_parameter("zb_out", [64, 2, 2, QC], F32, isOutput=True)
    with tile.TileContext(nc) as tc:
        _emit(nc, tc, qh, kh, vh, ckh, cvh, za_out, zb_out)
    nc.compile()
    _NC_CACHE = nc
    return nc


def make_in_maps(queries, keys, values, c_keys, c_values):
    in_maps = []
    for h in range(N_CORES):
        in_maps.append({
            "queries": np.ascontiguousarray(queries[0, h], dtype=np.float32),
            "keys": np.ascontiguousarray(keys[0, h], dtype=np.float32),
            "values": np.ascontiguousarray(values[0, h], dtype=np.float32),
            "c_keys": np.ascontiguousarray(c_keys[0, h], dtype=np.float32),
            "c_values": np.ascontiguousarray(c_values[0, h], dtype=np.float32),
        })
    return in_maps


def run_cores(in_maps, trace=False, **kw):
    nc = build_nc()
    return run_bass_kernel_spmd(nc, in_maps, list(range(N_CORES)),
                                trace=trace, **kw)


def _core_sq_err(r):
    """Sum of squared errors for one head from the z'/S dumps."""
    za = np.asarray(r["za_out"], dtype=np.float64)   # [65, 2, 2, 512]
    zb = np.asarray(r["zb_out"], dtype=np.float64)   # [64, 2, 2, 512]
    z = np.concatenate([za[0:64], zb], axis=0)       # [128d, phase, qc, 512]
    s = za[64]                                       # [phase, qc, 512]
    zn = z / s[None, :, :, :]
    d = zn[:, 1] - zn[:, 0]                          # teacher - compressed
    return float((d * d).sum())


def kernel(queries, keys, values, c_keys, c_values):
    res = run_cores(make_in_maps(queries, keys, values, c_keys, c_values))
    total = sum(_core_sq_err(r) for r in res.results)
    loss = total / float(B * H * Q * D)
    return np.asarray(loss, dtype=np.float32)


# revision 21
# speedup vs baseline: 1.0210x; 1.0210x over previous
"""Distillation-trainer loss kernel for Trainium2 (8 NeuronCores).

Computes  loss = mean((attn(q,k,v) - attn(q,ck,cv))**2)  for
q:[1,8,1024,128], k/v:[1,8,8192,128], ck/cv:[1,8,1024,128] fp32.

Sharding: one kv-head per core (h axis, 8 heads / 8 cores). Each core
returns its head's unnormalized attention outputs + softmax sums; the
host normalizes and reduces the scalar loss (the "all-reduce").

Per-core algorithm (head h):
  - K/CK/Q PE-transposed (bf16) to [d, n] / [d, q] layouts.
  - scoresT[n-tile, 0:1024] : stationary = kT tile, moving = full qT
    (2 matmuls of N=512). One LDWEIGHTS per n-tile amortized over 1024
    moving columns (v1 was weight-port-bound with 944 LDWEIGHTS).
  - exp on ACT -> fp8e4 probsT ring [n, q], one 1024-elem call per
    tile. exp bias -3.7 cancels in the softmax but keeps exp() < 224:
    the HW fp8e4 converter overflows to inf above ~240 (not 448 like
    ml_dtypes in CoreSim) and the max q.k/sqrt(d) score in this data
    is ~8.6 (dot-product tails are heavier than Gaussian).
  - PV in fp8 DoubleRow (2 n-tiles per matmul, contraction 256):
    stationary = V' [n, 2, 80] where cols 0:64 = V d-half, col 64 =
    ones (softmax denominator accumulates in PSUM row 64), 65:80 zero
    pad (DoubleRow k-tile byte step must be %16). probsT is the
    moving operand -- weight port stays far under the moving port.
  - unnormalized z' and S are DMA'd out; the host does z'/S and the
    MSE (on-device normalize cost ~26us of serial tail).
  - scheduling: PV lags one pair behind exp so the PE always refills
    the freed scores slot first (the exp chain never starves); K
    transposes are spread through the teacher loop at t%8==1.
"""

import numpy as np

import concourse.bass as bass
import concourse.mybir as mybir
import concourse.tile as tile
from concourse import bacc
from concourse.masks import make_identity
from concourse.bass_utils import run_bass_kernel_spmd

F32 = mybir.dt.float32
BF16 = mybir.dt.bfloat16
FP8 = mybir.dt.float8e4     # e4m3: PV operands (exp probs, values)
AF = mybir.ActivationFunctionType
ALU = mybir.AluOpType
DR = mybir.MatmulPerfMode.DoubleRow

B, H, Q, N, NC, D = 1, 8, 1024, 8192, 1024, 128
N_CORES = 8
SCALE = 1.0 / float(np.sqrt(D))
EXP_BIAS = -3.7

NT = N // 128               # 64 teacher n-tiles
NCT = NC // 128             # 8 compressed n-tiles
VW = 80                     # DoubleRow stationary width: 64 V + ones + pad
QC = 512                    # q chunk (PSUM bank = 512 fp32)
RING = 8                    # probsT ring depth (tiles)


def _emit(nc: bass.Bass, tc: tile.TileContext, qh, kh, vh, ckh, cvh,
          za_out, zb_out):
    ctxs = []

    def pool(**kw):
        p = tc.tile_pool(**kw)
        ctxs.append(p)
        return p.__enter__()

    pconst = pool(name="pconst", bufs=1)
    pstage = pool(name="pstage", bufs=2)
    psmall = pool(name="psmall", bufs=4)
    psc = pool(name="psc", bufs=2, space="PSUM")   # scores / transpose scratch
    ppv = pool(name="ppv", bufs=1, space="PSUM")   # PV accumulators

    # ---- persistent SBUF tensors ----
    ident = pconst.tile([128, 128], BF16, tag="ident")
    make_identity(nc, ident[:])

    qT = pconst.tile([128, Q], BF16, tag="qT")             # [d, q]
    kT = pconst.tile([128, NT, 128], BF16, tag="kT")       # [d, t, n]
    ckT = pconst.tile([128, NCT, 128], BF16, tag="ckT")
    va = pconst.tile([128, NT // 2, 2, VW], FP8, tag="va")   # V[:, :, 0:64]|1|0
    vb = pconst.tile([128, NT // 2, 2, VW], FP8, tag="vb")   # V[:, :, 64:128]|0
    cva = pconst.tile([128, NCT // 2, 2, VW], FP8, tag="cva")
    cvb = pconst.tile([128, NCT // 2, 2, VW], FP8, tag="cvb")
    ring = pconst.tile([128, RING, Q], FP8, tag="ring")    # probsT ring [n, q]

    for t_ in (va, cva):
        nc.gpsimd.memset(t_[:, :, :, 64:65], 1.0)
        nc.gpsimd.memset(t_[:, :, :, 65:VW], 0.0)
    for t_ in (vb, cvb):
        nc.gpsimd.memset(t_[:, :, :, 64:VW], 0.0)

    ebias = pconst.tile([128, 1], F32, tag="ebias")
    nc.gpsimd.memset(ebias[:], EXP_BIAS)

    # Warm the ACT exp table while prep DMAs run (~2.7us ACT_TABLE_LOAD).
    warm = psmall.tile([128, 1], F32, tag="warm")
    nc.gpsimd.memset(warm[:], 0.0)
    warm2 = psmall.tile([128, 1], F32, tag="warm2")
    nc.scalar.activation(warm2[:], warm[:], AF.Exp)

    # ---- loaders ----
    def load_kT_chunk(src, dst, g, tag):
        # 1024 rows -> cast bf16 -> 8 PE transposes -> dst[:, 8g:8g+8, :]
        stg = pstage.tile([128, 8, 128], F32, tag=tag)
        ap = src[g * 1024:(g + 1) * 1024, :].rearrange("(i p) d -> p i d", p=128)
        nc.sync.dma_start(out=stg[:], in_=ap)
        kb = pstage.tile([128, 8, 128], BF16, tag=tag + "b")
        nc.vector.tensor_copy(kb[:], stg[:])
        tp = psc.tile([128, 8, 128], BF16, tag="sc")
        for j in range(8):
            nc.tensor.transpose(tp[:, j, :], kb[:, j, :], ident[:])
        nc.vector.tensor_copy(dst[:, 8 * g:8 * g + 8, :], tp[:])

    def load_v_chunk(src, dsta, dstb, g, tag):
        # 1024 rows of V -> pairs 4g..4g+3, split d halves, cast to fp8.
        stg = pstage.tile([128, 8, 128], F32, tag=tag)
        ap = src[g * 1024:(g + 1) * 1024, :].rearrange("(i p) d -> p i d", p=128)
        nc.sync.dma_start(out=stg[:], in_=ap)
        sv = stg[:].rearrange("p (a b) d -> p a b d", b=2)  # [128, 4, 2, 128]
        nc.vector.tensor_copy(dsta[:, 4 * g:4 * g + 4, :, 0:64], sv[:, :, :, 0:64])
        nc.vector.tensor_copy(dstb[:, 4 * g:4 * g + 4, :, 0:64], sv[:, :, :, 64:128])

    load_kT_chunk(qh, qT[:].rearrange("p (i n) -> p i n", i=8), 0, "stq")
    load_kT_chunk(ckh, ckT, 0, "stck")
    load_v_chunk(cvh, cva, cvb, 0, "stcv")

    # K chunk DMAs into persistent fp32 staging; transposes happen later,
    # spread through the teacher loop. K/V interleaved so both arrive in
    # tile order on the serial DMA queue.
    kstg = pconst.tile([128, NT, 128], F32, tag="kstg")
    kb16 = pconst.tile([128, NT, 128], BF16, tag="kb16")

    def dma_k_chunk(g):
        kap = kh[g * 1024:(g + 1) * 1024, :].rearrange("(i p) d -> p i d", p=128)
        nc.sync.dma_start(out=kstg[:, 8 * g:8 * g + 8, :], in_=kap)
        nc.vector.tensor_copy(kb16[:, 8 * g:8 * g + 8, :], kstg[:, 8 * g:8 * g + 8, :])

    def transpose_k_chunk(g):
        tp = psc.tile([128, 8, 128], BF16, tag="sc")
        for j in range(8):
            nc.tensor.transpose(tp[:, j, :], kb16[:, 8 * g + j, :], ident[:])
        nc.vector.tensor_copy(kT[:, 8 * g:8 * g + 8, :], tp[:])

    for g in range(NT // 8):
        dma_k_chunk(g)
        load_v_chunk(vh, va, vb, g, "stv")

    # ---- PV accumulators (persist across one phase) ----
    za = [ppv.tile([128, QC], F32, tag=f"za{i}", name=f"za{i}") for i in range(2)]
    zb = [ppv.tile([128, QC], F32, tag=f"zb{i}", name=f"zb{i}") for i in range(2)]

    # Schraudolph fast-exp constants for the DVE offload path:
    # exp(s*SCALE + EXP_BIAS) ~ bitcast_f32(int32(A*s + B)). Max rel err
    # ~3%, comparable to the fp8e4 quantization the probs get anyway.
    LOG2E = 1.4426950408889634
    SCHRA_A = float(2.0 ** 23 * LOG2E * SCALE)
    SCHRA_B = float(2.0 ** 23 * (127 - 0.0436) + 2.0 ** 23 * LOG2E * EXP_BIAS)
    I32 = mybir.dt.int32

    def attend_tile(keysT, t, slot, dve_exp=False):
        sc = psc.tile([128, 2, QC], F32, tag="sc")
        nc.tensor.matmul(sc[:, 0, :], keysT[:, t, :], qT[:, 0:QC],
                         start=True, stop=True)
        nc.tensor.matmul(sc[:, 1, :], keysT[:, t, :], qT[:, QC:Q],
                         start=True, stop=True)
        if dve_exp:
            # exp on the (teacher-phase-idle) DVE so the ACT chain skips
            # this tile; only pass 1 holds the PSUM slot.
            ti = psmall.tile([128, Q], I32, tag="ti")
            nc.vector.tensor_scalar(ti[:], sc[:].rearrange("p a b -> p (a b)"),
                                    SCHRA_A, SCHRA_B, ALU.mult, ALU.add)
            nc.vector.tensor_copy(ring[:, slot, :], ti[:].bitcast(F32))
        else:
            ex = ring[:, slot, :].rearrange("p (a b) -> p a b", a=2)
            nc.scalar.activation(ex, sc[:], AF.Exp, scale=SCALE, bias=ebias[:])

    def pv_pair(vsa, vsb, p, n_pairs, slot0):
        st = dict(start=(p == 0), stop=(p == n_pairs - 1))
        rr = ring[:, slot0:slot0 + 2, :]   # [128, 2, 1024]
        for qc in range(2):
            mv = rr[:, :, qc * QC:(qc + 1) * QC]
            nc.tensor.matmul(za[qc][0:VW, :], vsa[:, p, :, :], mv,
                             perf_mode=DR, **st)
            nc.tensor.matmul(zb[qc][0:VW, :], vsb[:, p, :, :], mv,
                             perf_mode=DR, **st)

    def dump_phase(phase):
        # za rows 0:64 = z' d 0:64, row 64 = S; zb rows 0:64 = z' d 64:128
        for qc in range(2):
            da = psmall.tile([65, QC], F32, tag="da")
            nc.vector.tensor_copy(da[:], za[qc][0:65, :])
            nc.sync.dma_start(out=za_out[:, phase, qc, :], in_=da[:])
            db = psmall.tile([64, QC], F32, tag="db")
            nc.vector.tensor_copy(db[:], zb[qc][0:64, :])
            nc.sync.dma_start(out=zb_out[:, phase, qc, :], in_=db[:])

    # ---- Phase 1: compressed attention ----
    for t in range(NCT):
        attend_tile(ckT, t, t)
        if t % 2 == 1 and t >= 3:
            pv_pair(cva, cvb, (t - 2) // 2, NCT // 2, t - 3)
    pv_pair(cva, cvb, NCT // 2 - 1, NCT // 2, NCT - 2)
    transpose_k_chunk(0)
    dump_phase(0)

    # ---- Phase 2: teacher attention (PV lags one pair so the PE can
    # always refill the freed scores slot before stalling on exp) ----
    for t in range(NT):
        attend_tile(kT, t, t % RING)
        if t % 2 == 1 and t >= 3:
            pv_pair(va, vb, (t - 2) // 2, NT // 2, (t - 3) % RING)
        if t % 8 == 1 and t // 8 < 7:
            transpose_k_chunk(t // 8 + 1)
    pv_pair(va, vb, NT // 2 - 1, NT // 2, (NT - 2) % RING)
    dump_phase(1)

    for p in reversed(ctxs):
        p.__exit__(None, None, None)


_NC_CACHE = None


def build_nc():
    global _NC_CACHE
    if _NC_CACHE is not None:
        return _NC_CACHE
    nc = bacc.Bacc()
    qh = nc.declare_dram_parameter("queries", [Q, D], F32, isOutput=False)
    kh = nc.declare_dram_parameter("keys", [N, D], F32, isOutput=False)
    vh = nc.declare_dram_parameter("values", [N, D], F32, isOutput=False)
    ckh = nc.declare_dram_parameter("c_keys", [NC, D], F32, isOutput=False)
    cvh = nc.declare_dram_parameter("c_values", [NC, D], F32, isOutput=False)
    za_out = nc.declare_dram_parameter("za_out", [65, 2, 2, QC], F32, isOutput=True)
    zb_out = nc.declare_dram_parameter("zb_out", [64, 2, 2, QC], F32, isOutput=True)
    with tile.TileContext(nc) as tc:
        _emit(nc, tc, qh, kh, vh, ckh, cvh, za_out, zb_out)
    nc.compile()
    _NC_CACHE = nc
    return nc


def make_in_maps(queries, keys, values, c_keys, c_values):
    in_maps = []
    for h in range(N_CORES):
        in_maps.append({
            "queries": np.ascontiguousarray(queries[0, h], dtype=np.float32),
            "keys": np.ascontiguousarray(keys[0, h], dtype=np.float32),
            "values": np.ascontiguousarray(values[0, h], dtype=np.float32),
            "c_keys": np.ascontiguousarray(c_keys[0, h], dtype=np.float32),
            "c_values": np.ascontiguousarray(c_values[0, h], dtype=np.float32),
        })
    return in_maps


def run_cores(in_maps, trace=False, **kw):
    nc = build_nc()
    return run_bass_kernel_spmd(nc, in_maps, list(range(N_CORES)),
                                trace=trace, **kw)


def _core_sq_err(r):
    """Sum of squared errors for one head from the z'/S dumps."""
    za = np.asarray(r["za_out"], dtype=np.float64)   # [65, 2, 2, 512]
    zb = np.asarray(r["zb_out"], dtype=np.float64)   # [64, 2, 2, 512]
    z = np.concatenate([za[0:64], zb], axis=0)       # [128d, phase, qc, 512]
    s = za[64]                                       # [phase, qc, 512]
    zn = z / s[None, :, :, :]
    d = zn[:, 1] - zn[:, 0]                          # teacher - compressed
    return float((d * d).sum())


def kernel(queries, keys, values, c_keys, c_values):
    res = run_cores(make_in_maps(queries, keys, values, c_keys, c_values))
    total = sum(_core_sq_err(r) for r in res.results)
    loss = total / float(B * H * Q * D)
    return np.asarray(loss, dtype=np.float32)
